# revision 2
# baseline (speedup 1.0000x reference)
"""Longformer sliding-window self-attention (MBart variant) on 8 TRN2 cores, v4.

Sharding: core c -> (batch c//4, query quarter c%4). Each core gets a
halo'd transposed hidden-state slice [768, 1536], computes Q/K/V
projections, banded attention (4 chunks x 12 heads x 768-key windows),
and the output projection, returning [768, 1024].

v2 structural changes vs v1 (which measured ~390 us/core):
  - Phase 2 is a j-outer (head-pair) pipeline: PSUM = 2x 3-bank score
    tiles + 2x 1-bank ctx tiles = 8 banks, double-buffered across
    (n, hh) units so the PE never idles long enough to re-throttle.
  - Score matmuls are K=64 row-tiled ((0,0)/(64,0)) so the even/odd
    head matmuls can overlap in the PE array.
  - Both heads' ctx go through one [65, 512] PSUM tile at base 0 (ones
    column appended to V gives the softmax denominator on partition 64
    for both heads); the odd half reaches CT rows 64-127 via one
    [64, 1024] SBUF->SBUF DMA per j.
  - Softmax normalization is batched per j: denominator rows are
    broadcast to 64 partitions with a 0-stride-partition DMA, one
    reciprocal + one multiply per j. No per-(n,head) DMA/gpsimd hops.
  - exp outputs and V tiles are bf16 (half the SBUF, same PE rate).
  - All PSUM->SBUF evacuation runs on the Vector engine; the Scalar
    engine does only the 48 exp activations.
  - Band masks are per-chunk [NB, 128, 1024] (t0,t1 | t4,t5 halves
    only; t2,t3 are always fully in-band) applied with one strided
    DVE add per (n, head).
"""

import numpy as np

try:
    from ml_dtypes import bfloat16 as _bf16
except ImportError:  # pragma: no cover
    _bf16 = np.float16

# problem shapes (fixed by the task)
B, S, D, H = 2, 4096, 768, 12
DH = D // H            # 64
W = 256                # one-sided window == chunk size b
NEG = -1e9
NCORES = 8
G = 4                  # sequence groups per batch
SLOC = S // G          # 1024 queries per core
SH = SLOC + 2 * W      # 1536 halo'd rows per core
NB = SLOC // W         # 4 chunks per core
NT = 3 * W // 128      # 6 key tiles of 128 per chunk window
P = 128
DJ = D // P            # 6 tiles of 128 over the model dim

_PROGRAM_CACHE: dict = {}


def _build_program(general_mask: bool):
    import concourse.bacc as bacc
    import concourse.mybir as mybir
    import concourse.tile as tile
    from contextlib import ExitStack

    F32 = mybir.dt.float32
    F32R = mybir.dt.float32r
    BF16 = mybir.dt.bfloat16
    AF = mybir.ActivationFunctionType
    VAW = DH + 1                       # 65 cols per head in VA (dh + ones)
    MW = 1536 if general_mask else 1024  # mask cols per chunk

    nc = bacc.Bacc("TRN2", target_bir_lowering=False, debug=False)

    hsT = nc.dram_tensor("hsT", [D, SH], F32R, kind="ExternalInput")
    wqT = nc.dram_tensor("wqT", [D, D], F32R, kind="ExternalInput")
    wkT = nc.dram_tensor("wkT", [D, D], F32R, kind="ExternalInput")
    wvT = nc.dram_tensor("wvT", [D, D], F32R, kind="ExternalInput")
    woT = nc.dram_tensor("woT", [D, D], F32R, kind="ExternalInput")
    bq = nc.dram_tensor("bq", [D], F32, kind="ExternalInput")
    boe = nc.dram_tensor("boe", [D], F32, kind="ExternalInput")
    masks = nc.dram_tensor("masks", [NB, P, MW], BF16, kind="ExternalInput")
    outT = nc.dram_tensor("outT", [D, SLOC], F32, kind="ExternalOutput")

    with tile.TileContext(nc) as tc, ExitStack() as stack:
        const = stack.enter_context(tc.tile_pool(name="const", bufs=1))
        qt_p = stack.enter_context(tc.tile_pool(name="qt", bufs=1))
        kt_p = stack.enter_context(tc.tile_pool(name="kt", bufs=1))
        ct_p = stack.enter_context(tc.tile_pool(name="ct", bufs=1))
        va_p = stack.enter_context(tc.tile_pool(name="va", bufs=1))

        bq_sb = const.tile([P, DJ], F32, tag="bq")
        nc.gpsimd.dma_start(out=bq_sb[:], in_=bq.rearrange("(t p) -> p t", p=P))
        boe_sb = const.tile([P, DJ], F32, tag="boe")
        nc.gpsimd.dma_start(out=boe_sb[:], in_=boe.rearrange("(t p) -> p t", p=P))
        mask_sb = const.tile([P, NB * MW], BF16, tag="masks")
        for n in range(NB):
            nc.gpsimd.dma_start(out=mask_sb[:, n * MW : (n + 1) * MW], in_=masks[n])
        sel_e = const.tile([DH + 1, P], F32, tag="sel_e")
        nc.vector.memset(sel_e[:], 0.0)
        nc.vector.memset(sel_e[DH : DH + 1, 0:DH], 1.0)
        sel_o = const.tile([DH + 1, P], F32, tag="sel_o")
        nc.vector.memset(sel_o[:], 0.0)
        nc.vector.memset(sel_o[DH : DH + 1, DH:P], 1.0)

        QT = [qt_p.tile([P, SLOC], F32R, tag=f"qt{j}", name=f"qt{j}") for j in range(DJ)]
        KT = [kt_p.tile([P, SH], F32R, tag=f"kt{j}", name=f"kt{j}") for j in range(DJ)]
        CT = [ct_p.tile([P, SLOC], F32R, tag=f"ct{j}", name=f"ct{j}") for j in range(DJ)]
        VA = [va_p.tile([P, H * VAW], BF16, tag=f"va{s}", name=f"va{s}") for s in range(SH // P)]

        # ones column per head (bf16 1.0 exact)
        ones_f = const.tile([P, H], F32, tag="ones_f")
        nc.vector.memset(ones_f[:], 1.0)
        for st in range(SH // P):
            view = VA[st].rearrange("p (h e) -> p h e", e=VAW)
            nc.vector.tensor_copy(view[:, :, DH : DH + 1], ones_f[:])

        # ---------------- phase 1: projections ------------------------
        hs_stack = ExitStack()
        hs_p = hs_stack.enter_context(tc.tile_pool(name="hs", bufs=1))
        w_p = hs_stack.enter_context(tc.tile_pool(name="w", bufs=1))
        ps1 = hs_stack.enter_context(tc.tile_pool(name="ps1", bufs=4, space="PSUM"))
        HS = [hs_p.tile([P, SH], F32R, tag=f"hs{i}", name=f"hs{i}") for i in range(DJ)]
        for i in range(DJ):
            nc.sync.dma_start(out=HS[i][:], in_=hsT[P * i : P * (i + 1), :])

        for half in range(2):
            c0, c1 = 384 * half, 384 * (half + 1)
            # V = hs @ Wv.T -> VA (bf16, strided per-head layout)
            WV = [w_p.tile([P, 384], F32R, tag=f"wv{i}", name=f"wv{i}_{half}") for i in range(DJ)]
            for i in range(DJ):
                nc.scalar.dma_start(out=WV[i][:], in_=wvT[P * i : P * (i + 1), c0:c1])
            for sb in range(3):
                pss = [ps1.tile([P, 384], F32, tag="ps1", name=f"pv{half}_{sb}_{k}") for k in range(4)]
                for i in range(DJ):
                    for k in range(4):
                        st = 4 * sb + k
                        nc.tensor.matmul(
                            pss[k][:],
                            HS[i][:, P * st : P * (st + 1)],
                            WV[i][:],
                            start=(i == 0),
                            stop=(i == DJ - 1),
                        )
                for k in range(4):
                    st = 4 * sb + k
                    view = VA[st].rearrange("p (h e) -> p h e", e=VAW)
                    nc.vector.tensor_copy(
                        view[:, 6 * half : 6 * (half + 1), 0:DH],
                        pss[k][:].rearrange("p (h e) -> p h e", e=DH),
                    )

            # QT[j] = (Wq*scale) @ hs_loc.T + bq*scale
            WQ = [w_p.tile([P, 384], F32R, tag=f"wq{i}", name=f"wq{i}_{half}") for i in range(DJ)]
            for i in range(DJ):
                nc.scalar.dma_start(out=WQ[i][:], in_=wqT[P * i : P * (i + 1), c0:c1])
            for j in range(3 * half, 3 * half + 3):
                jc = P * j - c0
                for sp in range(SLOC // 512):
                    ps = ps1.tile([P, 512], F32, tag="ps1")
                    for i in range(DJ):
                        nc.tensor.matmul(
                            ps[:],
                            WQ[i][:, jc : jc + P],
                            HS[i][:, W + 512 * sp : W + 512 * (sp + 1)],
                            start=(i == 0),
                            stop=(i == DJ - 1),
                        )
                    nc.vector.tensor_scalar_add(
                        QT[j][:, 512 * sp : 512 * (sp + 1)], ps[:], bq_sb[:, j : j + 1]
                    )

            # KT[j] = Wk @ hs_halo.T (bias drops out of softmax)
            WK = [w_p.tile([P, 384], F32R, tag=f"wk{i}", name=f"wk{i}_{half}") for i in range(DJ)]
            for i in range(DJ):
                nc.scalar.dma_start(out=WK[i][:], in_=wkT[P * i : P * (i + 1), c0:c1])
            for j in range(3 * half, 3 * half + 3):
                jc = P * j - c0
                for sp in range(SH // 512):
                    ps = ps1.tile([P, 512], F32, tag="ps1")
                    for i in range(DJ):
                        nc.tensor.matmul(
                            ps[:],
                            WK[i][:, jc : jc + P],
                            HS[i][:, 512 * sp : 512 * (sp + 1)],
                            start=(i == 0),
                            stop=(i == DJ - 1),
                        )
                    nc.vector.tensor_copy(
                        KT[j][:, 512 * sp : 512 * (sp + 1)], ps[:]
                    )
        hs_stack.close()

        # ------------- phase 2 + 3: attention + output proj -----------
        with (
            tc.tile_pool(name="wo", bufs=1) as wo_p,
            tc.tile_pool(name="expp", bufs=1) as exp_p,
            tc.tile_pool(name="stg", bufs=2) as stg_p,
            tc.tile_pool(name="ob", bufs=3) as ob_p,
            tc.tile_pool(name="sps", bufs=2, space="PSUM") as sps_p,
            tc.tile_pool(name="cps", bufs=2, space="PSUM") as cps_p,
        ):
            WO = [wo_p.tile([P, D], F32R, tag=f"wo{i}", name=f"wo{i}") for i in range(DJ)]
            for i in range(DJ):
                nc.scalar.dma_start(out=WO[i][:], in_=woT[P * i : P * (i + 1), :])

            # persistent exp tiles: (hh, n%2) parity double-buffer
            ET = [
                [exp_p.tile([P, NT * W], BF16, tag=f"e{hh}{par}", name=f"e{hh}{par}")
                 for par in range(2)]
                for hh in range(2)
            ]

            for j in range(DJ):
                stgeo = stg_p.tile([DH + 1, 4 * 512], F32R, tag="stg", name=f"stg{j}")
                for n in range(NB):
                    cps = cps_p.tile([DH + 1, 512], F32, tag="cps", name=f"c{j}_{n}")
                    for hh in range(2):
                        r0 = DH * hh
                        sps = sps_p.tile([P, NT * W], F32, tag="sps", name=f"s{j}_{n}_{hh}")
                        for t in range(NT):
                            nc.tensor.matmul(
                                sps[:, W * t : W * (t + 1)],
                                KT[j][r0 : r0 + DH, W * n + P * t : W * n + P * (t + 1)],
                                QT[j][r0 : r0 + DH, W * n : W * (n + 1)],
                                start=True,
                                stop=True,
                            )
                        expt = ET[hh][n % 2]
                        nc.scalar.activation(expt[:], sps[:], AF.Exp)
                        # multiplicative band mask (bf16 SBUF): expt *= exp(mask)
                        if general_mask:
                            nc.vector.tensor_mul(
                                expt[:], expt[:], mask_sb[:, n * MW : (n + 1) * MW]
                            )
                        else:
                            ev = expt[:].rearrange("p (a q) -> p a q", a=3)[:, 0::2, :]
                            mv = mask_sb[:, n * MW : (n + 1) * MW].rearrange(
                                "p (a q) -> p a q", a=2
                            )
                            nc.vector.tensor_mul(ev, ev, mv)
                        # ctx + denominator (ones column -> partition 64)
                        for t in range(NT):
                            nc.tensor.matmul(
                                cps[:, W * hh : W * (hh + 1)],
                                VA[2 * n + t][:, VAW * (2 * j + hh) : VAW * (2 * j + hh + 1)],
                                expt[:, W * t : W * (t + 1)],
                                start=(t == 0),
                                stop=(t == NT - 1),
                            )
                    # one evacuation copy per (j, n): [65, 512] -> staging
                    nc.vector.tensor_copy(stgeo[:, 512 * n : 512 * (n + 1)], cps[:])
                # assemble CT halves from staging (strided SBUF->SBUF DMAs)
                sv = stgeo[0:DH, :].rearrange("p (n h q) -> p n h q", n=NB, h=2)
                nc.sync.dma_start(out=CT[j][0:DH, :], in_=sv[:, :, 0, :])
                nc.sync.dma_start(out=CT[j][DH:P, :], in_=sv[:, :, 1, :])
                # denominators: K=1 outer-product broadcast into PSUM, then divide
                dv = stgeo[0 : DH + 1, :].rearrange("p (n h q) -> p n h q", n=NB, h=2)
                for cb in range(2):
                    dn = cps_p.tile([P, 512], F32, tag="cps", name=f"dn{j}_{cb}")
                    nc.tensor.matmul(
                        dn[:],
                        sel_e[:].bitcast(F32R),
                        dv[:, 2 * cb : 2 * cb + 2, 0, :],
                        start=True,
                        stop=False,
                    )
                    nc.tensor.matmul(
                        dn[:],
                        sel_o[:].bitcast(F32R),
                        dv[:, 2 * cb : 2 * cb + 2, 1, :],
                        start=False,
                        stop=True,
                    )
                    nc.vector.reciprocal_approx_fast(out=dn[:], in_=dn[:])
                    nc.vector.tensor_mul(
                        CT[j][:, 512 * cb : 512 * (cb + 1)],
                        CT[j][:, 512 * cb : 512 * (cb + 1)],
                        dn[:],
                    )

            # ---------------- phase 3: output projection ---------------
            for jb in range(DJ):
                for sp in range(SLOC // 512):
                    pool3, tag3 = ((cps_p, "cps") if (2 * jb + sp) % 2 == 0
                                   else (sps_p, "sps"))
                    ps = pool3.tile([P, 512], F32, tag=tag3, name=f"o{jb}_{sp}")
                    for i in range(DJ):
                        nc.tensor.matmul(
                            ps[:],
                            WO[i][:, P * jb : P * (jb + 1)],
                            CT[i][:, 512 * sp : 512 * (sp + 1)],
                            start=(i == 0),
                            stop=(i == DJ - 1),
                        )
                    osb = ob_p.tile([P, 512], F32, tag="ob")
                    nc.vector.tensor_scalar_add(osb[:], ps[:], boe_sb[:, jb : jb + 1])
                    nc.sync.dma_start(
                        out=outT[P * jb : P * (jb + 1), 512 * sp : 512 * (sp + 1)],
                        in_=osb[:],
                    )

    nc.compile()
    return nc


def _host_prep(hidden_states, attention_mask, Wq, bq, Wk, bk, Wv, bv, Wo, bo):
    """Build per-core input maps. Returns (in_maps, general)."""
    hs = np.asarray(hidden_states, dtype=np.float32)
    am = np.asarray(attention_mask, dtype=np.float32)
    Wq = np.asarray(Wq, dtype=np.float32)
    Wk = np.asarray(Wk, dtype=np.float32)
    Wv = np.asarray(Wv, dtype=np.float32)
    Wo = np.asarray(Wo, dtype=np.float32)
    bq = np.asarray(bq, dtype=np.float32)
    bv = np.asarray(bv, dtype=np.float32)
    bo = np.asarray(bo, dtype=np.float32)

    general = bool(np.any(am != 0.0))
    scale = 1.0 / np.sqrt(np.float32(DH))

    wqT = np.ascontiguousarray(Wq.T * scale)
    wkT = np.ascontiguousarray(Wk.T)
    wvT = np.ascontiguousarray(Wv.T)
    woT = np.ascontiguousarray(Wo.T)
    bq_s = (bq * scale).astype(np.float32)
    bo_eff = (bo + Wo @ bv).astype(np.float32)

    # band validity per (tile t, partition p, q): kpos_w = 128 t + p
    t_idx = np.arange(NT)[:, None, None]
    p_idx = np.arange(P)[None, :, None]
    q_idx = np.arange(W)[None, None, :]
    kpos_w = P * t_idx + p_idx                      # [6,128,1]
    band_ok = np.abs(kpos_w - W - q_idx) <= W       # [6,128,256]

    def chunk_mask(gc, bi):
        """Full [6,128,256] additive mask for global chunk gc."""
        kglob = W * gc + kpos_w - W
        inb = (kglob >= 0) & (kglob < S)
        if general:
            kb = np.where(inb, -am[bi, np.clip(kglob, 0, S - 1)], 0.0)
        else:
            kb = np.zeros_like(kglob, dtype=np.float32)
        return np.where(band_ok & inb, kb, NEG).astype(np.float32)

    mt_int = None
    in_maps = []
    for c in range(NCORES):
        bi, g = divmod(c, G)
        lo = SLOC * g - W
        halo = np.zeros((SH, D), dtype=np.float32)
        s0, s1 = max(lo, 0), min(lo + SH, S)
        halo[s0 - lo : s1 - lo] = hs[bi, s0:s1]
        hsT_c = np.ascontiguousarray(halo.T)

        if general:
            m = np.empty((NB, P, 1536), dtype=np.float32)
            for n in range(NB):
                mt = chunk_mask(NB * g + n, bi)     # [6,128,256]
                m[n] = np.concatenate([mt[t] for t in range(NT)], axis=1)
        else:
            m = np.empty((NB, P, 1024), dtype=np.float32)
            if mt_int is None:
                mt = chunk_mask(5, 0)               # any interior chunk
                mt_int = np.concatenate([mt[0], mt[1], mt[4], mt[5]], axis=1)
            m[:] = mt_int[None]
            if g == 0:
                mt = chunk_mask(0, bi)
                m[0] = np.concatenate([mt[0], mt[1], mt[4], mt[5]], axis=1)
            elif g == G - 1:
                mt = chunk_mask(S // W - 1, bi)
                m[NB - 1] = np.concatenate([mt[0], mt[1], mt[4], mt[5]], axis=1)
        # multiplicative form: kernel multiplies exp(scores) by exp(mask)
        with np.errstate(under="ignore"):
            m = np.exp(m).astype(_bf16)

        in_maps.append(
            {
                "hsT": hsT_c,
                "wqT": wqT,
                "wkT": wkT,
                "wvT": wvT,
                "woT": woT,
                "bq": bq_s,
                "boe": bo_eff,
                "masks": m,
            }
        )
    return in_maps, general


def _run(inputs: dict, trace: bool = False):
    """Run the sharded kernel. Returns (full_output, BassKernelResults)."""
    from concourse.bass_utils import run_bass_kernel_spmd

    in_maps, general = _host_prep(**inputs)
    key = ("nc2", general)
    if key not in _PROGRAM_CACHE:
        _PROGRAM_CACHE[key] = _build_program(general)
    nc = _PROGRAM_CACHE[key]

    res = run_bass_kernel_spmd(nc, in_maps, list(range(NCORES)), trace=trace)
    out = np.empty((B, S, D), dtype=np.float32)
    for c in range(NCORES):
        bi, g = divmod(c, G)
        out[bi, SLOC * g : SLOC * (g + 1), :] = res.results[c]["outT"].T
    return out, res


def kernel(**inputs) -> np.ndarray:
    out, _ = _run(inputs, trace=False)
    return out


# revision 3
# speedup vs baseline: 1.0085x; 1.0085x over previous
"""Longformer sliding-window self-attention (MBart variant) on 8 TRN2 cores, v4.

Sharding: core c -> (batch c//4, query quarter c%4). Each core gets a
halo'd transposed hidden-state slice [768, 1536], computes Q/K/V
projections, banded attention (4 chunks x 12 heads x 768-key windows),
and the output projection, returning [768, 1024].

v2 structural changes vs v1 (which measured ~390 us/core):
  - Phase 2 is a j-outer (head-pair) pipeline: PSUM = 2x 3-bank score
    tiles + 2x 1-bank ctx tiles = 8 banks, double-buffered across
    (n, hh) units so the PE never idles long enough to re-throttle.
  - Score matmuls are K=64 row-tiled ((0,0)/(64,0)) so the even/odd
    head matmuls can overlap in the PE array.
  - Both heads' ctx go through one [65, 512] PSUM tile at base 0 (ones
    column appended to V gives the softmax denominator on partition 64
    for both heads); the odd half reaches CT rows 64-127 via one
    [64, 1024] SBUF->SBUF DMA per j.
  - Softmax normalization is batched per j: denominator rows are
    broadcast to 64 partitions with a 0-stride-partition DMA, one
    reciprocal + one multiply per j. No per-(n,head) DMA/gpsimd hops.
  - exp outputs and V tiles are bf16 (half the SBUF, same PE rate).
  - All PSUM->SBUF evacuation runs on the Vector engine; the Scalar
    engine does only the 48 exp activations.
  - Band masks are per-chunk [NB, 128, 1024] (t0,t1 | t4,t5 halves
    only; t2,t3 are always fully in-band) applied with one strided
    DVE add per (n, head).
"""

import numpy as np

try:
    from ml_dtypes import bfloat16 as _bf16
except ImportError:  # pragma: no cover
    _bf16 = np.float16

# problem shapes (fixed by the task)
B, S, D, H = 2, 4096, 768, 12
DH = D // H            # 64
W = 256                # one-sided window == chunk size b
NEG = -1e9
NCORES = 8
G = 4                  # sequence groups per batch
SLOC = S // G          # 1024 queries per core
SH = SLOC + 2 * W      # 1536 halo'd rows per core
NB = SLOC // W         # 4 chunks per core
NT = 3 * W // 128      # 6 key tiles of 128 per chunk window
P = 128
DJ = D // P            # 6 tiles of 128 over the model dim

_PROGRAM_CACHE: dict = {}


def _build_program(general_mask: bool):
    import concourse.bacc as bacc
    import concourse.mybir as mybir
    import concourse.tile as tile
    from contextlib import ExitStack

    F32 = mybir.dt.float32
    F32R = mybir.dt.float32r
    BF16 = mybir.dt.bfloat16
    AF = mybir.ActivationFunctionType
    VAW = DH + 1                       # 65 cols per head in VA (dh + ones)
    MW = 1536 if general_mask else 1024  # mask cols per chunk

    nc = bacc.Bacc("TRN2", target_bir_lowering=False, debug=False)

    hsT = nc.dram_tensor("hsT", [D, SH], F32R, kind="ExternalInput")
    wqT = nc.dram_tensor("wqT", [D, D], F32R, kind="ExternalInput")
    wkT = nc.dram_tensor("wkT", [D, D], F32R, kind="ExternalInput")
    wvT = nc.dram_tensor("wvT", [D, D], F32R, kind="ExternalInput")
    woT = nc.dram_tensor("woT", [D, D], F32R, kind="ExternalInput")
    bq = nc.dram_tensor("bq", [D], F32, kind="ExternalInput")
    boe = nc.dram_tensor("boe", [D], F32, kind="ExternalInput")
    masks = nc.dram_tensor("masks", [NB, P, MW], BF16, kind="ExternalInput")
    outT = nc.dram_tensor("outT", [D, SLOC], F32, kind="ExternalOutput")

    with tile.TileContext(nc) as tc, ExitStack() as stack:
        const = stack.enter_context(tc.tile_pool(name="const", bufs=1))
        qt_p = stack.enter_context(tc.tile_pool(name="qt", bufs=1))
        kt_p = stack.enter_context(tc.tile_pool(name="kt", bufs=1))
        ct_p = stack.enter_context(tc.tile_pool(name="ct", bufs=1))
        va_p = stack.enter_context(tc.tile_pool(name="va", bufs=1))

        bq_sb = const.tile([P, DJ], F32, tag="bq")
        nc.gpsimd.dma_start(out=bq_sb[:], in_=bq.rearrange("(t p) -> p t", p=P))
        boe_sb = const.tile([P, DJ], F32, tag="boe")
        nc.gpsimd.dma_start(out=boe_sb[:], in_=boe.rearrange("(t p) -> p t", p=P))
        mask_sb = const.tile([P, NB * MW], BF16, tag="masks")
        for n in range(NB):
            nc.gpsimd.dma_start(out=mask_sb[:, n * MW : (n + 1) * MW], in_=masks[n])
        sel_e = const.tile([DH + 1, P], F32, tag="sel_e")
        nc.vector.memset(sel_e[:], 0.0)
        nc.vector.memset(sel_e[DH : DH + 1, 0:DH], 1.0)
        sel_o = const.tile([DH + 1, P], F32, tag="sel_o")
        nc.vector.memset(sel_o[:], 0.0)
        nc.vector.memset(sel_o[DH : DH + 1, DH:P], 1.0)

        QT = [qt_p.tile([P, SLOC], F32R, tag=f"qt{j}", name=f"qt{j}") for j in range(DJ)]
        KT = [kt_p.tile([P, SH], F32R, tag=f"kt{j}", name=f"kt{j}") for j in range(DJ)]
        CT = [ct_p.tile([P, SLOC], F32R, tag=f"ct{j}", name=f"ct{j}") for j in range(DJ)]
        VA = [va_p.tile([P, H * VAW], BF16, tag=f"va{s}", name=f"va{s}") for s in range(SH // P)]

        # ones column per head (bf16 1.0 exact)
        ones_f = const.tile([P, H], F32, tag="ones_f")
        nc.vector.memset(ones_f[:], 1.0)
        for st in range(SH // P):
            view = VA[st].rearrange("p (h e) -> p h e", e=VAW)
            nc.vector.tensor_copy(view[:, :, DH : DH + 1], ones_f[:])

        # ---------------- phase 1: projections ------------------------
        hs_stack = ExitStack()
        hs_p = hs_stack.enter_context(tc.tile_pool(name="hs", bufs=1))
        w_p = hs_stack.enter_context(tc.tile_pool(name="w", bufs=1))
        ps1 = hs_stack.enter_context(tc.tile_pool(name="ps1", bufs=4, space="PSUM"))
        HS = [hs_p.tile([P, SH], F32R, tag=f"hs{i}", name=f"hs{i}") for i in range(DJ)]
        for i in range(DJ):
            nc.sync.dma_start(out=HS[i][:], in_=hsT[P * i : P * (i + 1), :])

        for half in range(2):
            c0, c1 = 384 * half, 384 * (half + 1)
            # V = hs @ Wv.T -> VA (bf16, strided per-head layout)
            WV = [w_p.tile([P, 384], F32R, tag=f"wv{i}", name=f"wv{i}_{half}") for i in range(DJ)]
            WQ = [w_p.tile([P, 384], F32R, tag=f"wq{i}", name=f"wq{i}_{half}") for i in range(DJ)]
            WK = [w_p.tile([P, 384], F32R, tag=f"wk{i}", name=f"wk{i}_{half}") for i in range(DJ)]
            for i in range(DJ):
                nc.scalar.dma_start(out=WV[i][:], in_=wvT[P * i : P * (i + 1), c0:c1])
            for i in range(DJ):
                nc.scalar.dma_start(out=WQ[i][:], in_=wqT[P * i : P * (i + 1), c0:c1])
            for i in range(DJ):
                nc.scalar.dma_start(out=WK[i][:], in_=wkT[P * i : P * (i + 1), c0:c1])
            for sb in range(3):
                pss = [ps1.tile([P, 384], F32, tag="ps1", name=f"pv{half}_{sb}_{k}") for k in range(4)]
                for i in range(DJ):
                    for k in range(4):
                        st = 4 * sb + k
                        nc.tensor.matmul(
                            pss[k][:],
                            HS[i][:, P * st : P * (st + 1)],
                            WV[i][:],
                            start=(i == 0),
                            stop=(i == DJ - 1),
                        )
                for k in range(4):
                    st = 4 * sb + k
                    view = VA[st].rearrange("p (h e) -> p h e", e=VAW)
                    nc.vector.tensor_copy(
                        view[:, 6 * half : 6 * (half + 1), 0:DH],
                        pss[k][:].rearrange("p (h e) -> p h e", e=DH),
                    )

            # QT[j] = (Wq*scale) @ hs_loc.T + bq*scale
            for j in range(3 * half, 3 * half + 3):
                jc = P * j - c0
                for sp in range(SLOC // 512):
                    ps = ps1.tile([P, 512], F32, tag="ps1")
                    for i in range(DJ):
                        nc.tensor.matmul(
                            ps[:],
                            WQ[i][:, jc : jc + P],
                            HS[i][:, W + 512 * sp : W + 512 * (sp + 1)],
                            start=(i == 0),
                            stop=(i == DJ - 1),
                        )
                    nc.vector.tensor_scalar_add(
                        QT[j][:, 512 * sp : 512 * (sp + 1)], ps[:], bq_sb[:, j : j + 1]
                    )

            # KT[j] = Wk @ hs_halo.T (bias drops out of softmax)
            for j in range(3 * half, 3 * half + 3):
                jc = P * j - c0
                for sp in range(SH // 512):
                    ps = ps1.tile([P, 512], F32, tag="ps1")
                    for i in range(DJ):
                        nc.tensor.matmul(
                            ps[:],
                            WK[i][:, jc : jc + P],
                            HS[i][:, 512 * sp : 512 * (sp + 1)],
                            start=(i == 0),
                            stop=(i == DJ - 1),
                        )
                    nc.vector.tensor_copy(
                        KT[j][:, 512 * sp : 512 * (sp + 1)], ps[:]
                    )
        hs_stack.close()

        # ------------- phase 2 + 3: attention + output proj -----------
        with (
            tc.tile_pool(name="wo", bufs=1) as wo_p,
            tc.tile_pool(name="expp", bufs=1) as exp_p,
            tc.tile_pool(name="stg", bufs=2) as stg_p,
            tc.tile_pool(name="ob", bufs=3) as ob_p,
            tc.tile_pool(name="sps", bufs=2, space="PSUM") as sps_p,
            tc.tile_pool(name="cps", bufs=2, space="PSUM") as cps_p,
        ):
            WO = [wo_p.tile([P, D], F32R, tag=f"wo{i}", name=f"wo{i}") for i in range(DJ)]
            for i in range(DJ):
                nc.scalar.dma_start(out=WO[i][:], in_=woT[P * i : P * (i + 1), :])

            # persistent exp tiles: (hh, n%2) parity double-buffer
            ET = [
                [exp_p.tile([P, NT * W], BF16, tag=f"e{hh}{par}", name=f"e{hh}{par}")
                 for par in range(2)]
                for hh in range(2)
            ]

            for j in range(DJ):
                stgeo = stg_p.tile([DH + 1, 4 * 512], F32R, tag="stg", name=f"stg{j}")
                for n in range(NB):
                    cps = cps_p.tile([DH + 1, 512], F32, tag="cps", name=f"c{j}_{n}")
                    for hh in range(2):
                        r0 = DH * hh
                        sps = sps_p.tile([P, NT * W], F32, tag="sps", name=f"s{j}_{n}_{hh}")
                        for t in range(NT):
                            nc.tensor.matmul(
                                sps[:, W * t : W * (t + 1)],
                                KT[j][r0 : r0 + DH, W * n + P * t : W * n + P * (t + 1)],
                                QT[j][r0 : r0 + DH, W * n : W * (n + 1)],
                                start=True,
                                stop=True,
                            )
                        expt = ET[hh][n % 2]
                        nc.scalar.activation(expt[:], sps[:], AF.Exp)
                        # multiplicative band mask (bf16 SBUF): expt *= exp(mask)
                        if general_mask:
                            nc.vector.tensor_mul(
                                expt[:], expt[:], mask_sb[:, n * MW : (n + 1) * MW]
                            )
                        else:
                            ev = expt[:].rearrange("p (a q) -> p a q", a=3)[:, 0::2, :]
                            mv = mask_sb[:, n * MW : (n + 1) * MW].rearrange(
                                "p (a q) -> p a q", a=2
                            )
                            nc.vector.tensor_mul(ev, ev, mv)
                        # ctx + denominator (ones column -> partition 64)
                        for t in range(NT):
                            nc.tensor.matmul(
                                cps[:, W * hh : W * (hh + 1)],
                                VA[2 * n + t][:, VAW * (2 * j + hh) : VAW * (2 * j + hh + 1)],
                                expt[:, W * t : W * (t + 1)],
                                start=(t == 0),
                                stop=(t == NT - 1),
                            )
                    # one evacuation copy per (j, n): [65, 512] -> staging
                    nc.vector.tensor_copy(stgeo[:, 512 * n : 512 * (n + 1)], cps[:])
                # assemble CT halves from staging (strided SBUF->SBUF DMAs)
                sv = stgeo[0:DH, :].rearrange("p (n h q) -> p n h q", n=NB, h=2)
                nc.sync.dma_start(out=CT[j][0:DH, :], in_=sv[:, :, 0, :])
                nc.sync.dma_start(out=CT[j][DH:P, :], in_=sv[:, :, 1, :])
                # denominators: K=1 outer-product broadcast into PSUM, then divide
                dv = stgeo[0 : DH + 1, :].rearrange("p (n h q) -> p n h q", n=NB, h=2)
                for cb in range(2):
                    dn = cps_p.tile([P, 512], F32, tag="cps", name=f"dn{j}_{cb}")
                    nc.tensor.matmul(
                        dn[:],
                        sel_e[:].bitcast(F32R),
                        dv[:, 2 * cb : 2 * cb + 2, 0, :],
                        start=True,
                        stop=False,
                    )
                    nc.tensor.matmul(
                        dn[:],
                        sel_o[:].bitcast(F32R),
                        dv[:, 2 * cb : 2 * cb + 2, 1, :],
                        start=False,
                        stop=True,
                    )
                    nc.vector.reciprocal_approx_fast(out=dn[:], in_=dn[:])
                    nc.vector.tensor_mul(
                        CT[j][:, 512 * cb : 512 * (cb + 1)],
                        CT[j][:, 512 * cb : 512 * (cb + 1)],
                        dn[:],
                    )

            # ---------------- phase 3: output projection ---------------
            for jb in range(DJ):
                for sp in range(SLOC // 512):
                    pool3, tag3 = ((cps_p, "cps") if (2 * jb + sp) % 2 == 0
                                   else (sps_p, "sps"))
                    ps = pool3.tile([P, 512], F32, tag=tag3, name=f"o{jb}_{sp}")
                    for i in range(DJ):
                        nc.tensor.matmul(
                            ps[:],
                            WO[i][:, P * jb : P * (jb + 1)],
                            CT[i][:, 512 * sp : 512 * (sp + 1)],
                            start=(i == 0),
                            stop=(i == DJ - 1),
                        )
                    osb = ob_p.tile([P, 512], F32, tag="ob")
                    nc.vector.tensor_scalar_add(osb[:], ps[:], boe_sb[:, jb : jb + 1])
                    nc.sync.dma_start(
                        out=outT[P * jb : P * (jb + 1), 512 * sp : 512 * (sp + 1)],
                        in_=osb[:],
                    )

    nc.compile()
    return nc


def _host_prep(hidden_states, attention_mask, Wq, bq, Wk, bk, Wv, bv, Wo, bo):
    """Build per-core input maps. Returns (in_maps, general)."""
    hs = np.asarray(hidden_states, dtype=np.float32)
    am = np.asarray(attention_mask, dtype=np.float32)
    Wq = np.asarray(Wq, dtype=np.float32)
    Wk = np.asarray(Wk, dtype=np.float32)
    Wv = np.asarray(Wv, dtype=np.float32)
    Wo = np.asarray(Wo, dtype=np.float32)
    bq = np.asarray(bq, dtype=np.float32)
    bv = np.asarray(bv, dtype=np.float32)
    bo = np.asarray(bo, dtype=np.float32)

    general = bool(np.any(am != 0.0))
    scale = 1.0 / np.sqrt(np.float32(DH))

    wqT = np.ascontiguousarray(Wq.T * scale)
    wkT = np.ascontiguousarray(Wk.T)
    wvT = np.ascontiguousarray(Wv.T)
    woT = np.ascontiguousarray(Wo.T)
    bq_s = (bq * scale).astype(np.float32)
    bo_eff = (bo + Wo @ bv).astype(np.float32)

    # band validity per (tile t, partition p, q): kpos_w = 128 t + p
    t_idx = np.arange(NT)[:, None, None]
    p_idx = np.arange(P)[None, :, None]
    q_idx = np.arange(W)[None, None, :]
    kpos_w = P * t_idx + p_idx                      # [6,128,1]
    band_ok = np.abs(kpos_w - W - q_idx) <= W       # [6,128,256]

    def chunk_mask(gc, bi):
        """Full [6,128,256] additive mask for global chunk gc."""
        kglob = W * gc + kpos_w - W
        inb = (kglob >= 0) & (kglob < S)
        if general:
            kb = np.where(inb, -am[bi, np.clip(kglob, 0, S - 1)], 0.0)
        else:
            kb = np.zeros_like(kglob, dtype=np.float32)
        return np.where(band_ok & inb, kb, NEG).astype(np.float32)

    mt_int = None
    in_maps = []
    for c in range(NCORES):
        bi, g = divmod(c, G)
        lo = SLOC * g - W
        halo = np.zeros((SH, D), dtype=np.float32)
        s0, s1 = max(lo, 0), min(lo + SH, S)
        halo[s0 - lo : s1 - lo] = hs[bi, s0:s1]
        hsT_c = np.ascontiguousarray(halo.T)

        if general:
            m = np.empty((NB, P, 1536), dtype=np.float32)
            for n in range(NB):
                mt = chunk_mask(NB * g + n, bi)     # [6,128,256]
                m[n] = np.concatenate([mt[t] for t in range(NT)], axis=1)
        else:
            m = np.empty((NB, P, 1024), dtype=np.float32)
            if mt_int is None:
                mt = chunk_mask(5, 0)               # any interior chunk
                mt_int = np.concatenate([mt[0], mt[1], mt[4], mt[5]], axis=1)
            m[:] = mt_int[None]
            if g == 0:
                mt = chunk_mask(0, bi)
                m[0] = np.concatenate([mt[0], mt[1], mt[4], mt[5]], axis=1)
            elif g == G - 1:
                mt = chunk_mask(S // W - 1, bi)
                m[NB - 1] = np.concatenate([mt[0], mt[1], mt[4], mt[5]], axis=1)
        # multiplicative form: kernel multiplies exp(scores) by exp(mask)
        with np.errstate(under="ignore"):
            m = np.exp(m).astype(_bf16)

        in_maps.append(
            {
                "hsT": hsT_c,
                "wqT": wqT,
                "wkT": wkT,
                "wvT": wvT,
                "woT": woT,
                "bq": bq_s,
                "boe": bo_eff,
                "masks": m,
            }
        )
    return in_maps, general


def _run(inputs: dict, trace: bool = False):
    """Run the sharded kernel. Returns (full_output, BassKernelResults)."""
    from concourse.bass_utils import run_bass_kernel_spmd

    in_maps, general = _host_prep(**inputs)
    key = ("nc2", general)
    if key not in _PROGRAM_CACHE:
        _PROGRAM_CACHE[key] = _build_program(general)
    nc = _PROGRAM_CACHE[key]

    res = run_bass_kernel_spmd(nc, in_maps, list(range(NCORES)), trace=trace)
    out = np.empty((B, S, D), dtype=np.float32)
    for c in range(NCORES):
        bi, g = divmod(c, G)
        out[bi, SLOC * g : SLOC * (g + 1), :] = res.results[c]["outT"].T
    return out, res


def kernel(**inputs) -> np.ndarray:
    out, _ = _run(inputs, trace=False)
    return out
